# revision 9
# baseline (speedup 1.0000x reference)
"""Trainium2 Bass kernel for nn_Decoder_49469433315669 (sparse_attention).

Tensor-parallel over heads: 32 heads / 8 cores = 4 heads per core.
Each core computes, for its 4 heads: Q/K/V projections (+RoPE), the
angle-LSH hash draft scores, raw true scores, causal softmax attention and
a partial output projection.  Host sums the o_proj partials (all-reduce)
and stacks the per-head score slices.

All matmuls run in float32r (TF32-like: full PE rate at free-dim >= 256,
~1e-4 relative error).  Layouts are transposed ([D, S], head-dim on
partitions) so no on-device transposes are needed anywhere.
"""
import numpy as np

import concourse.bass as bass
import concourse.mybir as mybir
import concourse.tile as tile
from concourse.alu_op_type import AluOpType
from concourse.vector_clock import ScopedClock
from concourse.bass_utils import run_bass_kernel_spmd

B, S, E, H, D = 1, 1024, 4096, 32, 128
NCORES = 8
HL = H // NCORES            # heads per core = 4
GAMMA = 64.0
ROPE_BASE = 10000.0
SCALE = 1.0 / np.sqrt(np.float32(D))
NEG = -1.0e9

dt = mybir.dt
AF = mybir.ActivationFunctionType


class _PatchedTC(tile.TileContext):
    """This walrus build rejects >1 sync wait on the kernel-tail drain;
    split the waits across several drain instructions."""

    MAX_WAITS = 1

    def _drain_and_barrier(self, tick_clock, wait_clock):
        drain_inst = self.nc.sync.drain()
        wait_clock.add_sem_waits(
            drain_inst.ins, ScopedClock({None: tick_clock.global_clock})
        )
        si = drain_inst.ins.sync_info
        if si is not None and si.on_wait and len(si.on_wait) > self.MAX_WAITS:
            waits = list(si.on_wait)
            si.on_wait = waits[: self.MAX_WAITS]
            rest = waits[self.MAX_WAITS :]
            for i in range(0, len(rest), self.MAX_WAITS):
                extra = self.nc.sync.drain()
                esi = extra.ins.sync_info
                chunk = rest[i : i + self.MAX_WAITS]
                if esi is None:
                    extra.ins.sync_info = mybir.SyncInfo(on_wait=chunk, on_update=[])
                else:
                    esi.on_wait = chunk
        self.nc.all_engine_barrier()
        popped = self.nc._tile_sem_poison_stack.pop()
        assert popped is self._sem_poison
        self.nc.clear_and_free_semaphores(list(self.sems.allocated().values()))
        self.nc.all_engine_barrier()


def _split_excess_waits(nc, max_waits=1):
    """Hoist excess per-instruction sync waits onto same-engine NoOps
    (engine streams are in-order, so semantics are preserved)."""
    ctr = 0
    for f in nc.m.functions:
        for bb in f.blocks:
            out = []
            changed = False
            for inst in bb.instructions:
                si = inst.sync_info
                if si is not None and si.on_wait and len(si.on_wait) > max_waits:
                    waits = list(si.on_wait)
                    rest, keep = waits[:-max_waits], waits[-max_waits:]
                    for i in range(0, len(rest), max_waits):
                        ctr += 1
                        nop = mybir.InstNoOp(name=f"WSPLIT-{ctr}", ins=[], outs=[])
                        nop.engine = inst.engine
                        nop.sync_info = mybir.SyncInfo(
                            on_wait=rest[i : i + max_waits], on_update=[]
                        )
                        out.append(nop)
                    si.on_wait = keep
                    changed = True
                out.append(inst)
            if changed:
                bb.instructions = out
    return ctr


def _build_program():
    nc = bass.Bass()
    f32, f32r = dt.float32, dt.float32r

    xT = nc.dram_tensor("xT", [E, S], f32r, kind="ExternalInput")
    wqT = nc.dram_tensor("wqT", [E, HL * D], f32r, kind="ExternalInput")
    wkT = nc.dram_tensor("wkT", [E, HL * D], f32r, kind="ExternalInput")
    wvT = nc.dram_tensor("wvT", [E, HL * D], f32r, kind="ExternalInput")
    woT = nc.dram_tensor("woT", [HL * D, E], f32r, kind="ExternalInput")
    proj0 = nc.dram_tensor("proj0", [D, HL * D], f32r, kind="ExternalInput")
    proj1 = nc.dram_tensor("proj1", [D, HL * D], f32r, kind="ExternalInput")
    bias0 = nc.dram_tensor("bias0", [D, HL], f32, kind="ExternalInput")
    bias1 = nc.dram_tensor("bias1", [D, HL], f32, kind="ExternalInput")
    cosT = nc.dram_tensor("cosT", [D, S], f32, kind="ExternalInput")
    sinT = nc.dram_tensor("sinT", [D, S], f32, kind="ExternalInput")
    negmask = nc.dram_tensor("negmask", [128, 128], f32, kind="ExternalInput")
    negmaskT = nc.dram_tensor("negmaskT", [4, 128, 512], f32, kind="ExternalInput")
    onesrow = nc.dram_tensor("onesrow", [1, 128], f32r, kind="ExternalInput")

    true_s = nc.dram_tensor("true_s", [HL, S, S], f32, kind="ExternalOutput")
    draft_s = nc.dram_tensor("draft_s", [HL, S, S], f32, kind="ExternalOutput")
    o_part = nc.dram_tensor("o_part", [S, E], f32, kind="ExternalOutput")
    dbg_z = nc.dram_tensor("dbg_z", [HL, 128, 8], f32, kind="ExternalOutput")
    dbg_zrow = nc.dram_tensor("dbg_zrow", [HL, 1, S], f32, kind="ExternalOutput")
    dbg_ctx = nc.dram_tensor("dbg_ctx", [HL, D, S], f32, kind="ExternalOutput")
    dbg_num = nc.dram_tensor("dbg_num", [HL, D, S], f32, kind="ExternalOutput")
    dbg_bcs = nc.dram_tensor("dbg_bcs", [HL, D, S], f32, kind="ExternalOutput")
    dbg_et = nc.dram_tensor("dbg_et", [8, 128, 512], f32, kind="ExternalOutput")

    NQT = S // 128   # 8 q-tiles
    NSC = S // 512   # 2 s-chunks

    with _PatchedTC(nc) as tc:
        from contextlib import ExitStack

        with ExitStack() as ctx:
            cpool = ctx.enter_context(tc.tile_pool(name="const", bufs=1))
            qkr = ctx.enter_context(tc.tile_pool(name="qkr", bufs=1))
            vpool = ctx.enter_context(tc.tile_pool(name="v", bufs=1))

            cos_sb = cpool.tile([D, S], f32)
            sin_sb = cpool.tile([D, S], f32)
            nc.sync.dma_start(cos_sb[:], cosT[:])
            nc.sync.dma_start(sin_sb[:], sinT[:])

            qrT = [qkr.tile([D, S], f32r, name=f"qrT{h}") for h in range(HL)]
            krT = [qkr.tile([D, S], f32r, name=f"krT{h}") for h in range(HL)]
            v_sb = [vpool.tile([128, HL * D], f32r, name=f"v{st}") for st in range(NQT)]

            # ---------------- Phase 1: QKV projections + fused RoPE ----------
            with (
                tc.tile_pool(name="xh", bufs=1) as xpool,
                tc.tile_pool(name="wstr", bufs=3) as wpool,
                tc.tile_pool(name="part", bufs=1) as ppool,
                tc.tile_pool(name="rtmp", bufs=2) as rpool,
                tc.tile_pool(name="ps1", bufs=8, space="PSUM") as ps1,
            ):
                # partial accumulators across the two k-halves
                part_q = [ppool.tile([128, 512], f32, name=f"pq{g}") for g in range(8)]
                part_k = [ppool.tile([128, 512], f32, name=f"pk{g}") for g in range(8)]
                part_v = [ppool.tile([128, 512], f32, name=f"pv{g}") for g in range(8)]

                def rope_copyback(psum, partial, dst, sc):
                    """dst[:, sc] = rope(psum + partial) (second k-half).
                    sin_sb is sign-baked: rows 0:64 hold -sin, rows 64:128
                    +sin, so rot-half reduces to a partition-swap DMA."""
                    cs = cos_sb[:, 512 * sc : 512 * (sc + 1)]
                    sn = sin_sb[:, 512 * sc : 512 * (sc + 1)]
                    s_ = rpool.tile([128, 512], f32, name="ropesum")
                    nc.vector.tensor_add(s_[:], psum[:], partial[:])
                    rot = rpool.tile([128, 512], f32, name="roperot")
                    nc.sync.dma_start(rot[0:64, :], s_[64:128, :])
                    nc.sync.dma_start(rot[64:128, :], s_[0:64, :])
                    t2 = rpool.tile([128, 512], f32, name="ropetmp2")
                    nc.vector.tensor_mul(t2[:], s_[:], cs[:])
                    nc.vector.tensor_mul(rot[:], rot[:], sn[:])
                    d_ = dst[:, 512 * sc : 512 * (sc + 1)]
                    nc.vector.tensor_add(d_[:], t2[:], rot[:])

                for kh in range(2):
                    xh = []
                    for t in range(4):
                        xt = xpool.tile([128, 4, S], f32r, name=f"xh{t}")
                        nc.sync.dma_start(
                            xt[:],
                            xT[2048 * kh + 512 * t : 2048 * kh + 512 * (t + 1), :]
                            .rearrange("(t p) s -> p t s", p=128),
                        )
                        xh.append(xt)

                    wq_sb, wk_sb, wv_sb = [], [], []
                    for k in range(16):
                        kk = 16 * kh + k
                        for wsrc, lst, nm in (
                            (wqT, wq_sb, "wq"),
                            (wkT, wk_sb, "wk"),
                            (wvT, wv_sb, "wv"),
                        ):
                            wt = wpool.tile([128, 512], f32r, name=nm)
                            nc.sync.dma_start(
                                wt[:], wsrc[128 * kk : 128 * (kk + 1), :]
                            )
                            lst.append(wt)

                    # segment A: Q-proj (8 groups), B: K-proj, C: V-proj
                    for w_sb, part, dstl in (
                        (wq_sb, part_q, qrT),
                        (wk_sb, part_k, krT),
                    ):
                        psg = [ps1.tile([128, 512], f32, name="ps1") for _ in range(8)]
                        for k in range(16):
                            for eo in range(4):
                                wl = w_sb[k][:, 128 * eo : 128 * (eo + 1)]
                                for sc in range(NSC):
                                    nc.tensor.matmul(
                                        psg[eo * 2 + sc][:],
                                        wl,
                                        xh[k // 4][:, k % 4, 512 * sc : 512 * (sc + 1)],
                                        start=(k == 0),
                                        stop=(k == 15),
                                    )
                        for eo in range(4):
                            for sc in range(NSC):
                                g = eo * 2 + sc
                                if kh == 0:
                                    nc.scalar.copy(part[g][:], psg[g][:])
                                else:
                                    rope_copyback(psg[g], part[g], dstl[eo], sc)

                    psv = [ps1.tile([128, 512], f32, name="ps1") for _ in range(8)]
                    for k in range(16):
                        for st in range(NQT):
                            nc.tensor.matmul(
                                psv[st][:],
                                xh[k // 4][:, k % 4, 128 * st : 128 * (st + 1)],
                                wv_sb[k][:],
                                start=(k == 0),
                                stop=(k == 15),
                            )
                    for st in range(NQT):
                        if kh == 0:
                            nc.scalar.copy(part_v[st][:], psv[st][:])
                        else:
                            nc.vector.tensor_add(v_sb[st][:], psv[st][:], part_v[st][:])

            # ---------------- Phase 2 constants ------------------------------
            c2pool = ctx.enter_context(tc.tile_pool(name="const2", bufs=1))
            p0_sb = c2pool.tile([D, HL * D], f32r)
            p1_sb = c2pool.tile([D, HL * D], f32r)
            b0_sb = c2pool.tile([D, HL], f32)
            b1_sb = c2pool.tile([D, HL], f32)
            nm_sb = c2pool.tile([128, 128], f32)
            nmT_sb = c2pool.tile([128, 4, 512], f32)
            ones_sb = c2pool.tile([1, 128], f32r)
            nc.sync.dma_start(p0_sb[:], proj0[:])
            nc.sync.dma_start(p1_sb[:], proj1[:])
            nc.sync.dma_start(b0_sb[:], bias0[:])
            nc.sync.dma_start(b1_sb[:], bias1[:])
            nc.sync.dma_start(nm_sb[:], negmask[:])
            nc.sync.dma_start(nmT_sb[:], negmaskT[:].rearrange("t p q -> p t q"))
            nc.sync.dma_start(ones_sb[:], onesrow[:])

            # ---------------- Phase 2a: hash + draft scores ------------------
            with (
                tc.tile_pool(name="hsh", bufs=2) as hpool,
                tc.tile_pool(name="dcp", bufs=3) as dpool,
                tc.tile_pool(name="ps2ah", bufs=2, space="PSUM") as ps2ah,
                tc.tile_pool(name="ps2ad", bufs=2, space="PSUM") as ps2ad,
            ):
                for h in range(HL):
                    p0h = p0_sb[:, 128 * h : 128 * (h + 1)]
                    p1h = p1_sb[:, 128 * h : 128 * (h + 1)]
                    b0h = b0_sb[:, h : h + 1]
                    b1h = b1_sb[:, h : h + 1]
                    hashed = []
                    for src in (qrT[h], krT[h]):
                        h1 = hpool.tile([128, S], f32r, name="h1")
                        for sc in range(NSC):
                            ps = ps2ah.tile([128, 512], f32, name="psh")
                            nc.tensor.matmul(
                                ps[:], p0h, src[:, 512 * sc : 512 * (sc + 1)],
                                start=True, stop=True,
                            )
                            s1 = hpool.tile([128, 512], f32, name="s1")
                            nc.scalar.activation(s1[:], ps[:], AF.Silu, bias=b0h)
                            nc.vector.tensor_add(
                                h1[:, 512 * sc : 512 * (sc + 1)], s1[:],
                                src[:, 512 * sc : 512 * (sc + 1)],
                            )
                        qh = hpool.tile([128, S], f32r, name="qh")
                        for sc in range(NSC):
                            ps = ps2ah.tile([128, 512], f32, name="psh")
                            nc.tensor.matmul(
                                ps[:], p1h, h1[:, 512 * sc : 512 * (sc + 1)],
                                start=True, stop=True,
                            )
                            h2 = hpool.tile([128, 512], f32, name="h2")
                            nc.vector.scalar_tensor_tensor(
                                h2[:], ps[:], b1h,
                                h1[:, 512 * sc : 512 * (sc + 1)],
                                AluOpType.add, AluOpType.add,
                            )
                            a_ = hpool.tile([128, 512], f32, name="abs")
                            nc.scalar.activation(a_[:], h2[:], AF.Abs, scale=GAMMA)
                            nc.vector.tensor_scalar_add(a_[:], a_[:], 1.0)
                            nc.vector.reciprocal(a_[:], a_[:])
                            nc.vector.tensor_mul(
                                qh[:, 512 * sc : 512 * (sc + 1)], h2[:], a_[:]
                            )
                        hashed.append(qh)
                    qh_t, kh_t = hashed
                    # draft scores: 4096 * qh'.kh'  ([q, k] orientation)
                    for qt in range(NQT):
                        psd = ps2ad.tile([128, S], f32, name="psd")
                        for kc in range(NSC):
                            nc.tensor.matmul(
                                psd[:, 512 * kc : 512 * (kc + 1)],
                                qh_t[:, 128 * qt : 128 * (qt + 1)],
                                kh_t[:, 512 * kc : 512 * (kc + 1)],
                                start=True, stop=True,
                            )
                        dsb = dpool.tile([128, S], f32, name="dsb")
                        nc.scalar.activation(
                            dsb[:], psd[:], AF.Copy, scale=GAMMA * GAMMA
                        )
                        nc.sync.dma_start(
                            draft_s[h, 128 * qt : 128 * (qt + 1), :], dsb[:]
                        )

            # ---------------- Phase 2b: true scores + softmax + PV -----------
            zpool = ctx.enter_context(tc.tile_pool(name="z", bufs=1))
            ctxp = ctx.enter_context(tc.tile_pool(name="ctx", bufs=1))
            ctx_sb = [ctxp.tile([D, S], f32r, name=f"ctx{h}") for h in range(HL)]
            with (
                tc.tile_pool(name="ssb", bufs=3) as spool,
                tc.tile_pool(name="esb", bufs=2) as epool,
                tc.tile_pool(name="etl", bufs=9) as etpool,
                tc.tile_pool(name="bcp", bufs=2) as bcpool,
                tc.tile_pool(name="ps2bs", bufs=2, space="PSUM") as ps2bs,
                tc.tile_pool(name="ps2bt", bufs=2, space="PSUM") as ps2bt,
                tc.tile_pool(name="ps2bc", bufs=1, space="PSUM") as ps2bc,
                tc.tile_pool(name="ps2bb", bufs=1, space="PSUM") as ps2bb,
            ):
                for h in range(HL):
                    zc = zpool.tile([128, NQT], f32, name=f"zc{h}")
                    zrow = zpool.tile([1, S], f32r, name=f"zrow{h}")
                    for qt in range(NQT):
                        pss = ps2bs.tile([128, S], f32, name="pss")
                        for kc in range(NSC):
                            nc.tensor.matmul(
                                pss[:, 512 * kc : 512 * (kc + 1)],
                                qrT[h][:, 128 * qt : 128 * (qt + 1)],
                                krT[h][:, 512 * kc : 512 * (kc + 1)],
                                start=True, stop=True,
                            )
                        ssb = spool.tile([128, S], f32, name="ssb")
                        nc.scalar.copy(ssb[:], pss[:])
                        nc.sync.dma_start(
                            true_s[h, 128 * qt : 128 * (qt + 1), :], ssb[:]
                        )
                        # causal mask on the diagonal block, then exp+rowsum
                        dg = pss[:, 128 * qt : 128 * (qt + 1)]
                        nc.vector.tensor_add(dg, dg, nm_sb[:])
                        esb = epool.tile([128, S], f32, name="esb")
                        ncols = 128 * (qt + 1)
                        nc.scalar.activation(
                            esb[:, 0:ncols], pss[:, 0:ncols], AF.Exp,
                            scale=float(SCALE), accum_out=zc[:, qt : qt + 1],
                        )
                    nc.sync.dma_start(dbg_z[h], zc[:])
                    zr = zpool.tile([128, NQT], f32, name=f"zr{h}")
                    nc.vector.reciprocal(zr[:], zc[:])
                    for qt in range(NQT):
                        nc.sync.dma_start(
                            zrow[0:1, 128 * qt : 128 * (qt + 1)],
                            zr[:, qt : qt + 1].bitcast(f32r),
                        )
                    for qc in range(NSC):
                        nk = 4 * qc + 4
                        psc = ps2bc.tile([128, 512], f32, name="psc")
                        ets = []
                        for j in range(nk):
                            pst = ps2bt.tile([128, 512], f32, name="pst")
                            nc.tensor.matmul(
                                pst[:],
                                krT[h][:, 128 * j : 128 * (j + 1)],
                                qrT[h][:, 512 * qc : 512 * (qc + 1)],
                                start=True, stop=True,
                            )
                            if j >= 4 * qc:
                                t = j - 4 * qc
                                w = 128 * (t + 1)
                                dgt = pst[:, 0:w]
                                nc.vector.tensor_add(dgt, dgt, nmT_sb[:, t, 0:w])
                            et = etpool.tile([128, 512], f32r, name="et")
                            nc.scalar.activation(
                                et[:], pst[:], AF.Exp, scale=float(SCALE)
                            )
                            if h == 0 and qc == 1:
                                nc.sync.dma_start(dbg_et[j], et[:].bitcast(f32))
                            ets.append(et)
                        for j in range(nk):
                            nc.tensor.matmul(
                                psc[:],
                                v_sb[j][:, 128 * h : 128 * (h + 1)],
                                ets[j][:],
                                start=(j == 0),
                                stop=(j == nk - 1),
                            )
                        ntmp = bcpool.tile([128, 512], f32, name="ntmp")
                        nc.scalar.copy(ntmp[:], psc[:])
                        nc.sync.dma_start(dbg_num[h, :, 512*qc:512*(qc+1)], ntmp[:])
                        psb = ps2bb.tile([128, 512], f32, name="psb")
                        nc.tensor.matmul(
                            psb[:], ones_sb[:],
                            zrow[0:1, 512 * qc : 512 * (qc + 1)],
                            start=True, stop=True,
                        )
                        bcs = bcpool.tile([128, 512], f32, name="bcs")
                        nc.scalar.copy(bcs[:], psb[:])
                        nc.sync.dma_start(dbg_bcs[h, :, 512*qc:512*(qc+1)], bcs[:])
                        nc.vector.tensor_mul(
                            ctx_sb[h][:, 512 * qc : 512 * (qc + 1)], psc[:], bcs[:]
                        )
                    nc.sync.dma_start(dbg_zrow[h], zrow[:].bitcast(f32))
                    nc.sync.dma_start(dbg_ctx[h], ctx_sb[h][:].bitcast(f32))

            # ---------------- Phase 3: output projection ---------------------
            with (
                tc.tile_pool(name="wo", bufs=8) as wopool,
                tc.tile_pool(name="osb", bufs=3) as opool,
                tc.tile_pool(name="ps3", bufs=2, space="PSUM") as ps3,
            ):
                for ec in range(8):
                    wo_sb = []
                    for h in range(HL):
                        wt = wopool.tile([128, 512], f32r, name="wo")
                        nc.sync.dma_start(
                            wt[:],
                            woT[128 * h : 128 * (h + 1), 512 * ec : 512 * (ec + 1)],
                        )
                        wo_sb.append(wt)
                    for st in range(NQT):
                        pso = ps3.tile([128, 512], f32, name="pso")
                        for h in range(HL):
                            nc.tensor.matmul(
                                pso[:],
                                ctx_sb[h][:, 128 * st : 128 * (st + 1)],
                                wo_sb[h][:],
                                start=(h == 0),
                                stop=(h == HL - 1),
                            )
                        osb = opool.tile([128, 512], f32, name="osb")
                        nc.scalar.copy(osb[:], pso[:])
                        nc.sync.dma_start(
                            o_part[
                                128 * st : 128 * (st + 1), 512 * ec : 512 * (ec + 1)
                            ],
                            osb[:],
                        )

    _split_excess_waits(nc)
    return nc


_NC = None


def _get_nc():
    global _NC
    if _NC is None:
        _NC = _build_program()
    return _NC


def _host_prep(inputs):
    hs = np.asarray(inputs["hidden_states"], np.float32)
    Wq = np.asarray(inputs["Wq"], np.float32)
    Wk = np.asarray(inputs["Wk"], np.float32)
    Wv = np.asarray(inputs["Wv"], np.float32)
    Wo = np.asarray(inputs["Wo"], np.float32)
    p0 = np.asarray(inputs["proj0"], np.float32)
    p1 = np.asarray(inputs["proj1"], np.float32)
    b0 = np.asarray(inputs["bias0"], np.float32)
    b1 = np.asarray(inputs["bias1"], np.float32)

    xT = np.ascontiguousarray(hs[0].T)  # [E, S]

    inv_freq = 1.0 / (ROPE_BASE ** (np.arange(0, D, 2, dtype=np.float32) / D))
    freqs = np.arange(S, dtype=np.float32)[:, None] * inv_freq[None, :]
    emb = np.concatenate([freqs, freqs], axis=-1)  # [S, D]
    cosT = np.ascontiguousarray(np.cos(emb).T.astype(np.float32))
    sinT = np.sin(emb).T.astype(np.float32)
    sinT[0 : D // 2, :] *= -1.0  # sign baked for the rot-half partition swap
    sinT = np.ascontiguousarray(sinT)

    qk = np.arange(128)
    nm = np.where(qk[None, :] <= qk[:, None], 0.0, NEG).astype(np.float32)  # [q,k]
    kk = np.arange(128)[:, None]
    cc = np.arange(512)[None, :]
    bb = cc // 128
    nmT = np.empty((4, 128, 512), np.float32)
    for t in range(4):
        full = np.where(bb < t, NEG, 0.0)
        diag = np.where((bb == t) & (kk > (cc % 128)), NEG, 0.0)
        nmT[t] = (full + diag).astype(np.float32)
    nmT = np.ascontiguousarray(nmT)
    onesrow = np.ones((1, 128), np.float32)

    in_maps = []
    for c in range(NCORES):
        sl = slice(512 * c, 512 * (c + 1))
        hsl = slice(HL * c, HL * (c + 1))
        in_maps.append(
            {
                "xT": xT,
                "wqT": np.ascontiguousarray(Wq[sl, :].T),
                "wkT": np.ascontiguousarray(Wk[sl, :].T),
                "wvT": np.ascontiguousarray(Wv[sl, :].T),
                "woT": np.ascontiguousarray(Wo[:, sl].T),
                "proj0": np.ascontiguousarray(
                    p0[0, hsl].transpose(1, 0, 2).reshape(D, HL * D)
                ),
                "proj1": np.ascontiguousarray(
                    p1[0, hsl].transpose(1, 0, 2).reshape(D, HL * D)
                ),
                "bias0": np.ascontiguousarray(b0[0, hsl, 0, :].T),
                "bias1": np.ascontiguousarray(b1[0, hsl, 0, :].T),
                "cosT": cosT,
                "sinT": sinT,
                "negmask": nm,
                "negmaskT": nmT,
                "onesrow": onesrow,
            }
        )
    return in_maps


def _assemble(results):
    true = np.empty((1, H, S, S), np.float32)
    draft = np.empty((1, H, S, S), np.float32)
    attn = np.zeros((S, E), np.float64)
    for c, r in enumerate(results):
        true[0, HL * c : HL * (c + 1)] = r["true_s"]
        draft[0, HL * c : HL * (c + 1)] = r["draft_s"]
        attn += r["o_part"].astype(np.float64)
    return attn.astype(np.float32).reshape(B, S, E), draft, true


def _run(inputs, profile_dir=None):
    nc = _get_nc()
    in_maps = _host_prep(inputs)
    res = run_bass_kernel_spmd(nc, in_maps, core_ids=list(range(NCORES)))
    out = _assemble(res.results)
    if profile_dir is not None:
        import os
        import shutil
        from trn_agent_boot.trn_boot import _ntff_profile_via_ctypes

        hook = _ntff_profile_via_ctypes("/opt/axon/libaxon_pjrt.so")
        shutil.rmtree(profile_dir, ignore_errors=True)
        os.makedirs(profile_dir, exist_ok=True)
        with hook(profile_dir, None):
            run_bass_kernel_spmd(nc, in_maps, core_ids=list(range(NCORES)))
    return out


def kernel(**inputs):
    attn, draft, true = _run(inputs)
    return attn, draft, true


# revision 10
# speedup vs baseline: 1.0140x; 1.0140x over previous
"""Trainium2 Bass kernel for nn_Decoder_49469433315669 (sparse_attention).

Tensor-parallel over heads: 32 heads / 8 cores = 4 heads per core.
Each core computes, for its 4 heads: Q/K/V projections (+RoPE), the
angle-LSH hash draft scores, raw true scores, causal softmax attention and
a partial output projection.  Host sums the o_proj partials (all-reduce)
and stacks the per-head score slices.

All matmuls run in float32r (TF32-like: full PE rate at free-dim >= 256,
~1e-4 relative error).  Layouts are transposed ([D, S], head-dim on
partitions) so no on-device transposes are needed anywhere.
"""
import numpy as np

import concourse.bass as bass
import concourse.mybir as mybir
import concourse.tile as tile
from concourse.alu_op_type import AluOpType
from concourse.vector_clock import ScopedClock
from concourse.bass_utils import run_bass_kernel_spmd

B, S, E, H, D = 1, 1024, 4096, 32, 128
NCORES = 8
HL = H // NCORES            # heads per core = 4
GAMMA = 64.0
ROPE_BASE = 10000.0
SCALE = 1.0 / np.sqrt(np.float32(D))
NEG = -1.0e9

dt = mybir.dt
AF = mybir.ActivationFunctionType


class _PatchedTC(tile.TileContext):
    """This walrus build rejects >1 sync wait on the kernel-tail drain;
    split the waits across several drain instructions."""

    MAX_WAITS = 1

    def _drain_and_barrier(self, tick_clock, wait_clock):
        drain_inst = self.nc.sync.drain()
        wait_clock.add_sem_waits(
            drain_inst.ins, ScopedClock({None: tick_clock.global_clock})
        )
        si = drain_inst.ins.sync_info
        if si is not None and si.on_wait and len(si.on_wait) > self.MAX_WAITS:
            waits = list(si.on_wait)
            si.on_wait = waits[: self.MAX_WAITS]
            rest = waits[self.MAX_WAITS :]
            for i in range(0, len(rest), self.MAX_WAITS):
                extra = self.nc.sync.drain()
                esi = extra.ins.sync_info
                chunk = rest[i : i + self.MAX_WAITS]
                if esi is None:
                    extra.ins.sync_info = mybir.SyncInfo(on_wait=chunk, on_update=[])
                else:
                    esi.on_wait = chunk
        self.nc.all_engine_barrier()
        popped = self.nc._tile_sem_poison_stack.pop()
        assert popped is self._sem_poison
        self.nc.clear_and_free_semaphores(list(self.sems.allocated().values()))
        self.nc.all_engine_barrier()


def _split_excess_waits(nc, max_waits=1):
    """Hoist excess per-instruction sync waits onto same-engine NoOps
    (engine streams are in-order, so semantics are preserved)."""
    ctr = 0
    for f in nc.m.functions:
        for bb in f.blocks:
            out = []
            changed = False
            for inst in bb.instructions:
                si = inst.sync_info
                if si is not None and si.on_wait and len(si.on_wait) > max_waits:
                    waits = list(si.on_wait)
                    rest, keep = waits[:-max_waits], waits[-max_waits:]
                    for i in range(0, len(rest), max_waits):
                        ctr += 1
                        nop = mybir.InstNoOp(name=f"WSPLIT-{ctr}", ins=[], outs=[])
                        nop.engine = inst.engine
                        nop.sync_info = mybir.SyncInfo(
                            on_wait=rest[i : i + max_waits], on_update=[]
                        )
                        out.append(nop)
                    si.on_wait = keep
                    changed = True
                out.append(inst)
            if changed:
                bb.instructions = out
    return ctr


def _build_program():
    nc = bass.Bass()
    f32, f32r = dt.float32, dt.float32r

    xT = nc.dram_tensor("xT", [E, S], f32r, kind="ExternalInput")
    wqT = nc.dram_tensor("wqT", [E, HL * D], f32r, kind="ExternalInput")
    wkT = nc.dram_tensor("wkT", [E, HL * D], f32r, kind="ExternalInput")
    wvT = nc.dram_tensor("wvT", [E, HL * D], f32r, kind="ExternalInput")
    woT = nc.dram_tensor("woT", [HL * D, E], f32r, kind="ExternalInput")
    proj0 = nc.dram_tensor("proj0", [D, HL * D], f32r, kind="ExternalInput")
    proj1 = nc.dram_tensor("proj1", [D, HL * D], f32r, kind="ExternalInput")
    bias0 = nc.dram_tensor("bias0", [D, HL], f32, kind="ExternalInput")
    bias1 = nc.dram_tensor("bias1", [D, HL], f32, kind="ExternalInput")
    cosT = nc.dram_tensor("cosT", [D, S], f32, kind="ExternalInput")
    sinT = nc.dram_tensor("sinT", [D, S], f32, kind="ExternalInput")
    negmask = nc.dram_tensor("negmask", [128, 128], f32, kind="ExternalInput")
    negmaskT = nc.dram_tensor("negmaskT", [4, 128, 512], f32, kind="ExternalInput")
    onesrow = nc.dram_tensor("onesrow", [1, 128], f32r, kind="ExternalInput")

    true_s = nc.dram_tensor("true_s", [HL, S, S], f32, kind="ExternalOutput")
    draft_s = nc.dram_tensor("draft_s", [HL, S, S], f32, kind="ExternalOutput")
    o_part = nc.dram_tensor("o_part", [S, E], f32, kind="ExternalOutput")

    NQT = S // 128   # 8 q-tiles
    NSC = S // 512   # 2 s-chunks

    with _PatchedTC(nc) as tc:
        from contextlib import ExitStack

        with ExitStack() as ctx:
            cpool = ctx.enter_context(tc.tile_pool(name="const", bufs=1))
            qkr = ctx.enter_context(tc.tile_pool(name="qkr", bufs=1))
            vpool = ctx.enter_context(tc.tile_pool(name="v", bufs=1))

            cos_sb = cpool.tile([D, S], f32)
            sin_sb = cpool.tile([D, S], f32)
            nc.sync.dma_start(cos_sb[:], cosT[:])
            nc.sync.dma_start(sin_sb[:], sinT[:])

            qrT = [qkr.tile([D, S], f32r, name=f"qrT{h}") for h in range(HL)]
            krT = [qkr.tile([D, S], f32r, name=f"krT{h}") for h in range(HL)]
            v_sb = [vpool.tile([128, HL * D], f32r, name=f"v{st}") for st in range(NQT)]

            # ---------------- Phase 1: QKV projections + fused RoPE ----------
            with (
                tc.tile_pool(name="xh", bufs=1) as xpool,
                tc.tile_pool(name="wstr", bufs=3) as wpool,
                tc.tile_pool(name="part", bufs=1) as ppool,
                tc.tile_pool(name="rtmp", bufs=2) as rpool,
                tc.tile_pool(name="ps1", bufs=8, space="PSUM") as ps1,
            ):
                # partial accumulators across the two k-halves
                part_q = [ppool.tile([128, 512], f32, name=f"pq{g}") for g in range(8)]
                part_k = [ppool.tile([128, 512], f32, name=f"pk{g}") for g in range(8)]
                part_v = [ppool.tile([128, 512], f32, name=f"pv{g}") for g in range(8)]

                def rope_copyback(psum, partial, dst, sc):
                    """dst[:, sc] = rope(psum + partial) (second k-half).
                    sin_sb is sign-baked: rows 0:64 hold -sin, rows 64:128
                    +sin, so rot-half reduces to a partition-swap DMA."""
                    cs = cos_sb[:, 512 * sc : 512 * (sc + 1)]
                    sn = sin_sb[:, 512 * sc : 512 * (sc + 1)]
                    s_ = rpool.tile([128, 512], f32, name="ropesum")
                    nc.vector.tensor_add(s_[:], psum[:], partial[:])
                    rot = rpool.tile([128, 512], f32, name="roperot")
                    nc.sync.dma_start(rot[0:64, :], s_[64:128, :])
                    nc.sync.dma_start(rot[64:128, :], s_[0:64, :])
                    t2 = rpool.tile([128, 512], f32, name="ropetmp2")
                    nc.vector.tensor_mul(t2[:], s_[:], cs[:])
                    nc.vector.tensor_mul(rot[:], rot[:], sn[:])
                    d_ = dst[:, 512 * sc : 512 * (sc + 1)]
                    nc.vector.tensor_add(d_[:], t2[:], rot[:])

                for kh in range(2):
                    xh = []
                    for t in range(4):
                        xt = xpool.tile([128, 4, S], f32r, name=f"xh{t}")
                        nc.sync.dma_start(
                            xt[:],
                            xT[2048 * kh + 512 * t : 2048 * kh + 512 * (t + 1), :]
                            .rearrange("(t p) s -> p t s", p=128),
                        )
                        xh.append(xt)

                    wq_sb, wk_sb, wv_sb = [], [], []
                    for k in range(16):
                        kk = 16 * kh + k
                        for wsrc, lst, nm in (
                            (wqT, wq_sb, "wq"),
                            (wkT, wk_sb, "wk"),
                            (wvT, wv_sb, "wv"),
                        ):
                            wt = wpool.tile([128, 512], f32r, name=nm)
                            nc.sync.dma_start(
                                wt[:], wsrc[128 * kk : 128 * (kk + 1), :]
                            )
                            lst.append(wt)

                    # segment A: Q-proj (8 groups), B: K-proj, C: V-proj
                    for w_sb, part, dstl in (
                        (wq_sb, part_q, qrT),
                        (wk_sb, part_k, krT),
                    ):
                        psg = [ps1.tile([128, 512], f32, name="ps1") for _ in range(8)]
                        for k in range(16):
                            for eo in range(4):
                                wl = w_sb[k][:, 128 * eo : 128 * (eo + 1)]
                                for sc in range(NSC):
                                    nc.tensor.matmul(
                                        psg[eo * 2 + sc][:],
                                        wl,
                                        xh[k // 4][:, k % 4, 512 * sc : 512 * (sc + 1)],
                                        start=(k == 0),
                                        stop=(k == 15),
                                    )
                        for eo in range(4):
                            for sc in range(NSC):
                                g = eo * 2 + sc
                                if kh == 0:
                                    nc.scalar.copy(part[g][:], psg[g][:])
                                else:
                                    rope_copyback(psg[g], part[g], dstl[eo], sc)

                    psv = [ps1.tile([128, 512], f32, name="ps1") for _ in range(8)]
                    for k in range(16):
                        for st in range(NQT):
                            nc.tensor.matmul(
                                psv[st][:],
                                xh[k // 4][:, k % 4, 128 * st : 128 * (st + 1)],
                                wv_sb[k][:],
                                start=(k == 0),
                                stop=(k == 15),
                            )
                    for st in range(NQT):
                        if kh == 0:
                            nc.scalar.copy(part_v[st][:], psv[st][:])
                        else:
                            nc.vector.tensor_add(v_sb[st][:], psv[st][:], part_v[st][:])

            # ---------------- Phase 2 constants ------------------------------
            c2pool = ctx.enter_context(tc.tile_pool(name="const2", bufs=1))
            p0_sb = c2pool.tile([D, HL * D], f32r)
            p1_sb = c2pool.tile([D, HL * D], f32r)
            b0_sb = c2pool.tile([D, HL], f32)
            b1_sb = c2pool.tile([D, HL], f32)
            nm_sb = c2pool.tile([128, 128], f32)
            nmT_sb = c2pool.tile([128, 4, 512], f32)
            ones_sb = c2pool.tile([1, 128], f32r)
            nc.sync.dma_start(p0_sb[:], proj0[:])
            nc.sync.dma_start(p1_sb[:], proj1[:])
            nc.sync.dma_start(b0_sb[:], bias0[:])
            nc.sync.dma_start(b1_sb[:], bias1[:])
            nc.sync.dma_start(nm_sb[:], negmask[:])
            nc.sync.dma_start(nmT_sb[:], negmaskT[:].rearrange("t p q -> p t q"))
            nc.sync.dma_start(ones_sb[:], onesrow[:])

            # ---------------- Phase 2a: hash + draft scores ------------------
            with (
                tc.tile_pool(name="hsh", bufs=2) as hpool,
                tc.tile_pool(name="dcp", bufs=3) as dpool,
                tc.tile_pool(name="ps2ah", bufs=2, space="PSUM") as ps2ah,
                tc.tile_pool(name="ps2ad", bufs=2, space="PSUM") as ps2ad,
            ):
                for h in range(HL):
                    p0h = p0_sb[:, 128 * h : 128 * (h + 1)]
                    p1h = p1_sb[:, 128 * h : 128 * (h + 1)]
                    b0h = b0_sb[:, h : h + 1]
                    b1h = b1_sb[:, h : h + 1]
                    hashed = []
                    for src in (qrT[h], krT[h]):
                        h1 = hpool.tile([128, S], f32r, name="h1")
                        for sc in range(NSC):
                            ps = ps2ah.tile([128, 512], f32, name="psh")
                            nc.tensor.matmul(
                                ps[:], p0h, src[:, 512 * sc : 512 * (sc + 1)],
                                start=True, stop=True,
                            )
                            s1 = hpool.tile([128, 512], f32, name="s1")
                            nc.scalar.activation(s1[:], ps[:], AF.Silu, bias=b0h)
                            nc.vector.tensor_add(
                                h1[:, 512 * sc : 512 * (sc + 1)], s1[:],
                                src[:, 512 * sc : 512 * (sc + 1)],
                            )
                        qh = hpool.tile([128, S], f32r, name="qh")
                        for sc in range(NSC):
                            ps = ps2ah.tile([128, 512], f32, name="psh")
                            nc.tensor.matmul(
                                ps[:], p1h, h1[:, 512 * sc : 512 * (sc + 1)],
                                start=True, stop=True,
                            )
                            h2 = hpool.tile([128, 512], f32, name="h2")
                            nc.vector.scalar_tensor_tensor(
                                h2[:], ps[:], b1h,
                                h1[:, 512 * sc : 512 * (sc + 1)],
                                AluOpType.add, AluOpType.add,
                            )
                            a_ = hpool.tile([128, 512], f32, name="abs")
                            nc.scalar.activation(a_[:], h2[:], AF.Abs, scale=GAMMA)
                            nc.vector.tensor_scalar_add(a_[:], a_[:], 1.0)
                            nc.vector.reciprocal(a_[:], a_[:])
                            nc.vector.tensor_mul(
                                qh[:, 512 * sc : 512 * (sc + 1)], h2[:], a_[:]
                            )
                        hashed.append(qh)
                    qh_t, kh_t = hashed
                    # draft scores: 4096 * qh'.kh'  ([q, k] orientation)
                    for qt in range(NQT):
                        psd = ps2ad.tile([128, S], f32, name="psd")
                        for kc in range(NSC):
                            nc.tensor.matmul(
                                psd[:, 512 * kc : 512 * (kc + 1)],
                                qh_t[:, 128 * qt : 128 * (qt + 1)],
                                kh_t[:, 512 * kc : 512 * (kc + 1)],
                                start=True, stop=True,
                            )
                        dsb = dpool.tile([128, S], f32, name="dsb")
                        nc.scalar.activation(
                            dsb[:], psd[:], AF.Copy, scale=GAMMA * GAMMA
                        )
                        nc.sync.dma_start(
                            draft_s[h, 128 * qt : 128 * (qt + 1), :], dsb[:]
                        )

            # ---------------- Phase 2b: true scores + softmax + PV -----------
            zpool = ctx.enter_context(tc.tile_pool(name="z", bufs=1))
            ctxp = ctx.enter_context(tc.tile_pool(name="ctx", bufs=1))
            ctx_sb = [ctxp.tile([D, S], f32r, name=f"ctx{h}") for h in range(HL)]
            with (
                tc.tile_pool(name="ssb", bufs=3) as spool,
                tc.tile_pool(name="esb", bufs=2) as epool,
                tc.tile_pool(name="etl", bufs=9) as etpool,
                tc.tile_pool(name="bcp", bufs=2) as bcpool,
                tc.tile_pool(name="ps2bs", bufs=2, space="PSUM") as ps2bs,
                tc.tile_pool(name="ps2bt", bufs=2, space="PSUM") as ps2bt,
                tc.tile_pool(name="ps2bc", bufs=1, space="PSUM") as ps2bc,
                tc.tile_pool(name="ps2bb", bufs=1, space="PSUM") as ps2bb,
            ):
                for h in range(HL):
                    zc = zpool.tile([128, NQT], f32, name=f"zc{h}")
                    zrow = zpool.tile([1, S], f32r, name=f"zrow{h}")
                    for qt in range(NQT):
                        pss = ps2bs.tile([128, S], f32, name="pss")
                        for kc in range(NSC):
                            nc.tensor.matmul(
                                pss[:, 512 * kc : 512 * (kc + 1)],
                                qrT[h][:, 128 * qt : 128 * (qt + 1)],
                                krT[h][:, 512 * kc : 512 * (kc + 1)],
                                start=True, stop=True,
                            )
                        ssb = spool.tile([128, S], f32, name="ssb")
                        nc.scalar.copy(ssb[:], pss[:])
                        nc.sync.dma_start(
                            true_s[h, 128 * qt : 128 * (qt + 1), :], ssb[:]
                        )
                        # causal mask on the diagonal block, then exp+rowsum
                        dg = pss[:, 128 * qt : 128 * (qt + 1)]
                        nc.vector.tensor_add(dg, dg, nm_sb[:])
                        esb = epool.tile([128, S], f32, name="esb")
                        ncols = 128 * (qt + 1)
                        nc.scalar.activation(
                            esb[:, 0:ncols], pss[:, 0:ncols], AF.Exp,
                            scale=float(SCALE), accum_out=zc[:, qt : qt + 1],
                        )
                    zr = zpool.tile([128, NQT], f32, name=f"zr{h}")
                    nc.vector.reciprocal(zr[:], zc[:])
                    for qt in range(NQT):
                        nc.sync.dma_start(
                            zrow[0:1, 128 * qt : 128 * (qt + 1)],
                            zr[:, qt : qt + 1].bitcast(f32r),
                        )
                    for qc in range(NSC):
                        nk = 4 * qc + 4
                        psc = ps2bc.tile([128, 512], f32, name="psc")
                        ets = []
                        for j in range(nk):
                            pst = ps2bt.tile([128, 512], f32, name="pst")
                            nc.tensor.matmul(
                                pst[:],
                                krT[h][:, 128 * j : 128 * (j + 1)],
                                qrT[h][:, 512 * qc : 512 * (qc + 1)],
                                start=True, stop=True,
                            )
                            if j >= 4 * qc:
                                t = j - 4 * qc
                                w = 128 * (t + 1)
                                dgt = pst[:, 0:w]
                                nc.vector.tensor_add(dgt, dgt, nmT_sb[:, t, 0:w])
                            et = etpool.tile([128, 512], f32r, name="et")
                            nc.scalar.activation(
                                et[:], pst[:], AF.Exp, scale=float(SCALE)
                            )
                            ets.append(et)
                        for j in range(nk):
                            nc.tensor.matmul(
                                psc[:],
                                v_sb[j][:, 128 * h : 128 * (h + 1)],
                                ets[j][:],
                                start=(j == 0),
                                stop=(j == nk - 1),
                            )
                        psb = ps2bb.tile([128, 512], f32, name="psb")
                        nc.tensor.matmul(
                            psb[:], ones_sb[:],
                            zrow[0:1, 512 * qc : 512 * (qc + 1)],
                            start=True, stop=True,
                        )
                        bcs = bcpool.tile([128, 512], f32, name="bcs")
                        nc.scalar.copy(bcs[:], psb[:])
                        nc.vector.tensor_mul(
                            ctx_sb[h][:, 512 * qc : 512 * (qc + 1)], psc[:], bcs[:]
                        )

            # ---------------- Phase 3: output projection ---------------------
            with (
                tc.tile_pool(name="wo", bufs=8) as wopool,
                tc.tile_pool(name="osb", bufs=3) as opool,
                tc.tile_pool(name="ps3", bufs=2, space="PSUM") as ps3,
            ):
                for ec in range(8):
                    wo_sb = []
                    for h in range(HL):
                        wt = wopool.tile([128, 512], f32r, name="wo")
                        nc.sync.dma_start(
                            wt[:],
                            woT[128 * h : 128 * (h + 1), 512 * ec : 512 * (ec + 1)],
                        )
                        wo_sb.append(wt)
                    for st in range(NQT):
                        pso = ps3.tile([128, 512], f32, name="pso")
                        for h in range(HL):
                            nc.tensor.matmul(
                                pso[:],
                                ctx_sb[h][:, 128 * st : 128 * (st + 1)],
                                wo_sb[h][:],
                                start=(h == 0),
                                stop=(h == HL - 1),
                            )
                        osb = opool.tile([128, 512], f32, name="osb")
                        nc.scalar.copy(osb[:], pso[:])
                        nc.sync.dma_start(
                            o_part[
                                128 * st : 128 * (st + 1), 512 * ec : 512 * (ec + 1)
                            ],
                            osb[:],
                        )

    _split_excess_waits(nc)
    return nc


_NC = None


def _get_nc():
    global _NC
    if _NC is None:
        _NC = _build_program()
    return _NC


def _host_prep(inputs):
    hs = np.asarray(inputs["hidden_states"], np.float32)
    Wq = np.asarray(inputs["Wq"], np.float32)
    Wk = np.asarray(inputs["Wk"], np.float32)
    Wv = np.asarray(inputs["Wv"], np.float32)
    Wo = np.asarray(inputs["Wo"], np.float32)
    p0 = np.asarray(inputs["proj0"], np.float32)
    p1 = np.asarray(inputs["proj1"], np.float32)
    b0 = np.asarray(inputs["bias0"], np.float32)
    b1 = np.asarray(inputs["bias1"], np.float32)

    xT = np.ascontiguousarray(hs[0].T)  # [E, S]

    inv_freq = 1.0 / (ROPE_BASE ** (np.arange(0, D, 2, dtype=np.float32) / D))
    freqs = np.arange(S, dtype=np.float32)[:, None] * inv_freq[None, :]
    emb = np.concatenate([freqs, freqs], axis=-1)  # [S, D]
    cosT = np.ascontiguousarray(np.cos(emb).T.astype(np.float32))
    sinT = np.sin(emb).T.astype(np.float32)
    sinT[0 : D // 2, :] *= -1.0  # sign baked for the rot-half partition swap
    sinT = np.ascontiguousarray(sinT)

    qk = np.arange(128)
    nm = np.where(qk[None, :] <= qk[:, None], 0.0, NEG).astype(np.float32)  # [q,k]
    kk = np.arange(128)[:, None]
    cc = np.arange(512)[None, :]
    bb = cc // 128
    nmT = np.empty((4, 128, 512), np.float32)
    for t in range(4):
        full = np.where(bb < t, NEG, 0.0)
        diag = np.where((bb == t) & (kk > (cc % 128)), NEG, 0.0)
        nmT[t] = (full + diag).astype(np.float32)
    nmT = np.ascontiguousarray(nmT)
    onesrow = np.ones((1, 128), np.float32)

    in_maps = []
    for c in range(NCORES):
        sl = slice(512 * c, 512 * (c + 1))
        hsl = slice(HL * c, HL * (c + 1))
        in_maps.append(
            {
                "xT": xT,
                "wqT": np.ascontiguousarray(Wq[sl, :].T),
                "wkT": np.ascontiguousarray(Wk[sl, :].T),
                "wvT": np.ascontiguousarray(Wv[sl, :].T),
                "woT": np.ascontiguousarray(Wo[:, sl].T),
                "proj0": np.ascontiguousarray(
                    p0[0, hsl].transpose(1, 0, 2).reshape(D, HL * D)
                ),
                "proj1": np.ascontiguousarray(
                    p1[0, hsl].transpose(1, 0, 2).reshape(D, HL * D)
                ),
                "bias0": np.ascontiguousarray(b0[0, hsl, 0, :].T),
                "bias1": np.ascontiguousarray(b1[0, hsl, 0, :].T),
                "cosT": cosT,
                "sinT": sinT,
                "negmask": nm,
                "negmaskT": nmT,
                "onesrow": onesrow,
            }
        )
    return in_maps


def _assemble(results):
    true = np.empty((1, H, S, S), np.float32)
    draft = np.empty((1, H, S, S), np.float32)
    attn = np.zeros((S, E), np.float64)
    for c, r in enumerate(results):
        true[0, HL * c : HL * (c + 1)] = r["true_s"]
        draft[0, HL * c : HL * (c + 1)] = r["draft_s"]
        attn += r["o_part"].astype(np.float64)
    return attn.astype(np.float32).reshape(B, S, E), draft, true


def _run(inputs, profile_dir=None):
    nc = _get_nc()
    in_maps = _host_prep(inputs)
    res = run_bass_kernel_spmd(nc, in_maps, core_ids=list(range(NCORES)))
    out = _assemble(res.results)
    if profile_dir is not None:
        import os
        import shutil
        from trn_agent_boot.trn_boot import _ntff_profile_via_ctypes

        hook = _ntff_profile_via_ctypes("/opt/axon/libaxon_pjrt.so")
        shutil.rmtree(profile_dir, ignore_errors=True)
        os.makedirs(profile_dir, exist_ok=True)
        with hook(profile_dir, None):
            run_bass_kernel_spmd(nc, in_maps, core_ids=list(range(NCORES)))
    return out


def kernel(**inputs):
    attn, draft, true = _run(inputs)
    return attn, draft, true
